# revision 28
# baseline (speedup 1.0000x reference)
"""DRASI encoder (MLP -> GraphConv x2 -> mu/logvar heads) on 8 Trainium2 cores.

Sharding: nodes are split into 8 contiguous shards of 6250. Each core runs the
node-local MLP on its shard, the shards are AllGathered into a full
[50000, 128] bf16 feature table in DRAM, and each core processes the edges
whose destination lies in its shard.

Aggregation strategy (both conv layers share the same edge structure):
  - edges are split by source-node parity (even/odd). The parity split is
    symmetric across cores, so unified (max-over-core) block counts carry
    minimal padding, and src//2 indices fit int16; each parity stream
    gathers through an elem_step=256 strided view of the table.
  - within each (chunk, parity) stream, edges are packed densely into
    128-edge gather blocks with no per-group padding; a block whose edges
    span a dst-group boundary contributes one masked matmul per group it
    touches. The selection matrices S_w[e, s] = w_e * (seg_e == s) are
    precomputed on the host in bf16 and streamed via plain DMA on the SP
    queue (which does not contend with the SWDGE gather path).
  - per-group PE matmuls accumulate aggT = msg.T @ S_w in PSUM (64-node
    groups) and evict to an SBUF table in bf16, alternating between the
    Act and DVE engines.
  - the GraphConv linear phase runs interleaved with aggregation: one PSUM
    group accumulates rel_W @ aggT + root_W @ h directly, one bias+relu
    activation per 512-column tile.
  - block counts and matmul sets are unified across cores (max/union) so all
    8 cores share one program.

Outputs (mu, logvar) are computed per shard and concatenated on the host.
"""
import sys
sys.path.insert(0, '/opt/trn_rl_repo')

import numpy as np
import concourse.bass as bass
import concourse.bacc as bacc
import concourse.mybir as mybir
from concourse.tile import TileContext
from concourse.masks import make_identity
from concourse import bass_utils

P = 128
N_CORES = 8
N_NODES = 50000
IN_DIM = 512
HID = 128
LAT = 32
SHARD = N_NODES // N_CORES          # 6250
HALF = N_NODES // 2                 # 25000
W = 64                              # nodes per segment group (PSUM tile width)
N_GROUPS = (SHARD + W - 1) // W     # 98 (last group 42 wide)
N_TILES = [512] * (SHARD // 512) + ([SHARD % 512] if SHARD % 512 else [])
F32 = mybir.dt.float32
BF16 = mybir.dt.bfloat16
I16 = mybir.dt.int16
import ml_dtypes
NP_BF16 = ml_dtypes.bfloat16


# ---------------------------------------------------------------- host prep --

def _idx_tile(idx_flat):
    """SWDGE index layout: [128, nblk*8] int16 from flat [nblk*128]."""
    nblk = idx_flat.shape[0] // P
    return np.tile(idx_flat.reshape(nblk * 8, 16).T, (8, 1))


def _structure(per_core_edges):
    """per_core_edges: list of (src_global, dst_local, w) sorted by dst_local.

    Returns (metas, eidx, esw); metas is a list of chunk dicts:
      groups: (gs, ge)
      nblk_ev, nblk_od
      mms: [(g, [(cat, b), ...]), ...] in emission order
    """
    cats = []           # per core: dict cat -> (gather idx, dstl, w)
    for c, (src, dstl, wgt) in enumerate(per_core_edges):
        mev = src % 2 == 0
        cats.append({
            'ev': (src[mev] // 2, dstl[mev], wgt[mev]),
            'od': (src[~mev] // 2, dstl[~mev], wgt[~mev]),
        })

    starts = [{k: np.searchsorted(v[1], np.arange(N_GROUPS + 1) * W)
               for k, v in cc.items()} for cc in cats]

    def nblk_range(cat, gs, ge):
        return max(int(np.ceil((starts[c][cat][ge] - starts[c][cat][gs]) / P))
                   for c in range(N_CORES))

    # greedy chunking with a decreasing tail so the post-gather drain
    # (last matmuls -> linear tiles -> publish, gating the AllGather) is
    # short; chunk size has no effect on the gather stream itself, which
    # runs back-to-back regardless
    def total_blocks(gs, ge):
        return nblk_range('ev', gs, ge) + nblk_range('od', gs, ge)

    whole = total_blocks(0, N_GROUPS)

    def cap_for(idx, emitted):
        rem = whole - emitted
        if rem > 150:
            return 84
        if rem > 110:
            return 60
        if rem > 70:
            return 40
        if rem > 40:
            return 24
        return 14

    chunks = []
    gs = 0
    emitted = 0
    for g in range(1, N_GROUPS + 1):
        cap = cap_for(len(chunks), emitted)
        nb = total_blocks(gs, g)
        if nb > cap and g - 1 > gs:
            chunks.append((gs, g - 1))
            emitted += total_blocks(gs, g - 1)
            gs = g - 1
    if gs < N_GROUPS:
        chunks.append((gs, N_GROUPS))

    metas = []
    idxs = [[] for _ in range(N_CORES)]
    sws = [[] for _ in range(N_CORES)]

    def cat_layout(cat, gs, ge):
        nblk = nblk_range(cat, gs, ge)
        mms = {}
        core_idx = []
        core_rows = []   # per core: dict (g, b) -> (slots, segs, ws)
        for c in range(N_CORES):
            st = starts[c][cat]
            base = st[gs]
            ids, dstl, ws = cats[c][cat]
            iflat = np.zeros(nblk * P, np.int16)
            n = st[ge] - base
            if n:
                iflat[:n] = ids[base:st[ge]].astype(np.int16)
            core_idx.append(iflat)
            rows = {}
            for g in range(gs, ge):
                e0, e1 = st[g] - base, st[g + 1] - base
                if e1 <= e0:
                    continue
                b0, b1 = e0 // P, (e1 - 1) // P
                mms.setdefault(g, set()).update(range(b0, b1 + 1))
                for b in range(b0, b1 + 1):
                    r0, r1 = max(e0, b * P), min(e1, (b + 1) * P)
                    sl = np.arange(r0, r1) - b * P
                    rows[(g, b)] = (sl,
                                    dstl[base + r0:base + r1] - g * W,
                                    ws[base + r0:base + r1])
            core_rows.append(rows)
        mms = {g: sorted(bs) for g, bs in mms.items()}
        return nblk, mms, core_idx, core_rows

    def sw_block(core_rows_c, g, b):
        sw = np.zeros((P, W), np.float32)
        ent = core_rows_c.get((g, b))
        if ent is not None:
            sl, segs, ws = ent
            sw[sl, segs] = ws
        return sw.astype(NP_BF16)

    for (gs, ge) in chunks:
        nbev, mmsev, idxev, rowsev = cat_layout('ev', gs, ge)
        nbod, mmsod, idxod, rowsod = cat_layout('od', gs, ge)

        gset = sorted(set(mmsev) | set(mmsod))
        mms = [(g, [('ev', b) for b in mmsev.get(g, [])]
                + [('od', b) for b in mmsod.get(g, [])]) for g in gset]

        metas.append(dict(groups=(gs, ge), nblk_ev=nbev, nblk_od=nbod,
                          mms=mms))

        for c in range(N_CORES):
            idxs[c].append(_idx_tile(np.concatenate([idxev[c], idxod[c]])))
            for g, ents in mms:
                for cat, b in ents:
                    sws[c].append(sw_block(
                        rowsev[c] if cat == 'ev' else rowsod[c], g, b))

    eidx = [np.ascontiguousarray(np.concatenate(idxs[c], axis=1))
            for c in range(N_CORES)]
    esw = [np.ascontiguousarray(np.concatenate(sws[c], axis=1))
           for c in range(N_CORES)]
    return metas, eidx, esw


# ------------------------------------------------------------- device build --

def _build(metas, dims):
    nc = bacc.Bacc(None, target_bir_lowering=False, num_devices=N_CORES,
                   num_swdge_queues=2)

    xT = nc.dram_tensor("xT", [IN_DIM, SHARD], BF16, kind="ExternalInput")
    w1T = nc.dram_tensor("w1T", [IN_DIM, HID], BF16, kind="ExternalInput")
    b1 = nc.dram_tensor("b1", [HID, 1], F32, kind="ExternalInput")
    w2T = nc.dram_tensor("w2T", [HID, HID], BF16, kind="ExternalInput")
    b2 = nc.dram_tensor("b2", [HID, 1], F32, kind="ExternalInput")
    conv_wT = nc.dram_tensor("conv_wT", [2, 2, HID, HID], BF16, kind="ExternalInput")
    conv_b = nc.dram_tensor("conv_b", [2, HID, 1], F32, kind="ExternalInput")
    headWT = nc.dram_tensor("headWT", [HID, 2 * LAT], BF16, kind="ExternalInput")
    head_b = nc.dram_tensor("head_b", [2 * LAT, 1], F32, kind="ExternalInput")
    eidx = nc.dram_tensor("eidx", [P, dims['idx']], I16, kind="ExternalInput")
    esw = nc.dram_tensor("esw", [P, dims['sw']], BF16, kind="ExternalInput")
    muv_out = nc.dram_tensor("muvT", [2 * LAT, SHARD], F32, kind="ExternalOutput")

    ag_in = [nc.dram_tensor(f"ag_in{i}", [SHARD, HID], BF16) for i in range(2)]
    tables = [nc.dram_tensor(f"h_full{i}", [N_NODES, HID], BF16,
                             addr_space="Shared") for i in range(2)]

    NT_FULL = SHARD // P            # 48 full 128-row publish tiles
    TAIL = SHARD - NT_FULL * P      # 106
    MAXBLK = max(m['nblk_ev'] + m['nblk_od'] for m in metas)
    MAXMM = max(sum(len(e) for _, e in m['mms']) for m in metas)

    with TileContext(nc) as tc:
        with (
            tc.tile_pool(name="const", bufs=1) as cp,
            tc.tile_pool(name="big", bufs=1) as bigp,
            tc.tile_pool(name="xsp", bufs=3) as xsp,
            tc.tile_pool(name="xact", bufs=3) as xact,
            tc.tile_pool(name="h1p", bufs=3) as h1p,
            tc.tile_pool(name="msgp", bufs=3) as msgp,
            tc.tile_pool(name="swp", bufs=2) as swp,
            tc.tile_pool(name="stg", bufs=3) as stg,
            tc.tile_pool(name="ps_tr", bufs=2, space="PSUM") as ps_tr,
        ):
            # ---- constants; MLP-critical ones on SP first (tiny), the rest
            # on the Pool DMA channel which is otherwise idle during the MLP
            w1t_sb = [cp.tile([P, HID], BF16, tag=f"w1_{k}", name=f"w1t_{k}")
                      for k in range(4)]
            b1_sb = cp.tile([P, 1], F32, tag="b1")
            b2_sb = cp.tile([P, 1], F32, tag="b2")
            w2t_sb = cp.tile([P, HID], BF16, tag="w2")
            # spread the MLP-critical consts across the three DMA queues so
            # the first x tiles aren't delayed
            nc.sync.dma_start(out=w1t_sb[0][:], in_=w1T[0:P, :])
            nc.sync.dma_start(out=b1_sb[:], in_=b1[:, :])
            nc.scalar.dma_start(out=w1t_sb[1][:], in_=w1T[P:2 * P, :])
            nc.gpsimd.dma_start(out=w1t_sb[2][:], in_=w1T[2 * P:3 * P, :])
            nc.gpsimd.dma_start(out=w1t_sb[3][:], in_=w1T[3 * P:4 * P, :])
            nc.gpsimd.dma_start(out=w2t_sb[:], in_=w2T[:, :])
            nc.gpsimd.dma_start(out=b2_sb[:], in_=b2[:, :])
            cw_sb = [[cp.tile([P, HID], BF16, tag=f"cw{l}{m}", name=f"cw_{l}_{m}")
                      for m in range(2)] for l in range(2)]
            cb_sb = [cp.tile([P, 1], F32, tag=f"cb{l}", name=f"cb_{l}")
                     for l in range(2)]
            for l in range(2):
                for m in range(2):
                    nc.gpsimd.dma_start(out=cw_sb[l][m][:], in_=conv_wT[l, m, :, :])
                nc.gpsimd.dma_start(out=cb_sb[l][:], in_=conv_b[l, :, :])
            hw_sb = cp.tile([P, 2 * LAT], BF16, tag="hw")
            nc.gpsimd.dma_start(out=hw_sb[:], in_=headWT[:, :])
            hb_sb = cp.tile([2 * LAT, 1], F32, tag="hb")
            nc.gpsimd.dma_start(out=hb_sb[:], in_=head_b[:, :])
            ident = cp.tile([P, P], BF16, tag="ident")
            make_identity(nc, ident[:])

            hA = bigp.tile([P, SHARD], BF16, tag="hA")   # h2T, then h4T
            hB = bigp.tile([P, SHARD], BF16, tag="hB")   # h3T
            aggT = bigp.tile([P, SHARD], BF16, tag="aggT")
            natf = [bigp.tile([P, NT_FULL, HID], BF16, tag=f"natf{i}",
                              name=f"natf_{i}") for i in range(2)]
            natt = [bigp.tile([P, HID], BF16, tag=f"natt{i}",
                              name=f"natt_{i}") for i in range(2)]
            eidx_sb = bigp.tile([P, dims['idx']], I16, tag="eidx")
            # edge index table: both layers reuse it; Pool channel is idle
            # during the MLP so the load is free there
            nc.gpsimd.dma_start(out=eidx_sb[:], in_=eidx[:, :])

            def emit_transpose_tiles(hT_tile, t_idx, n0, n1):
                t = n0 // P
                while n0 < n1:
                    w_ = min(P, n1 - n0)
                    tr_ps = ps_tr.tile([P, P], BF16, space="PSUM", tag="tr",
                                       name="trp")
                    nc.tensor.transpose(out=tr_ps[:w_, :],
                                        in_=hT_tile[:, n0:n0 + w_],
                                        identity=ident[:])
                    dst = natf[t_idx][:, t, :] if w_ == P else natt[t_idx][:TAIL, :]
                    nc.vector.tensor_copy(
                        out=dst[:w_, :] if w_ == P else dst,
                        in_=tr_ps[:w_, :])
                    n0 += w_
                    t += 1

            def emit_publish_piece(t_idx, c0, c1, eng):
                eng.dma_start(
                    out=ag_in[t_idx][c0 * P:c1 * P, :].rearrange(
                        "(t r) h -> r t h", r=P),
                    in_=natf[t_idx][:, c0:c1, :])
                if c1 == NT_FULL:
                    eng.dma_start(out=ag_in[t_idx][NT_FULL * P:, :],
                                  in_=natt[t_idx][:TAIL, :])

            PUB_AT = {4: (0, 16), 8: (16, 32), 11: (32, 40),
                      12: (40, 44), 13: (44, NT_FULL)}

            def maybe_publish(t_idx, done_tiles):
                if done_tiles in PUB_AT:
                    c0, c1 = PUB_AT[done_tiles]
                    # during the MLP (t_idx 0) SP is idle once the x tiles
                    # are in; during the conv layer (t_idx 1) SP streams S_w,
                    # so publishes ride Act instead (its channel is idle)
                    eng = nc.sync if t_idx == 0 else nc.scalar
                    emit_publish_piece(t_idx, c0, c1, eng)

            def emit_allgather(t_idx):
                nc.gpsimd.collective_compute(
                    "AllGather", mybir.AluOpType.bypass,
                    replica_groups=[list(range(N_CORES))],
                    ins=[ag_in[t_idx][:, :]],
                    outs=[tables[t_idx][:, :]],
                )

            # ---- MLP (bf16 matmuls, f32 psum), software-pipelined ----
            cols = [sum(N_TILES[:i]) for i in range(len(N_TILES))]
            h1_sbs = {}
            mlp_ps_cm = tc.tile_pool(name="mlp_ps", bufs=4, space="PSUM")
            mlp_ps = mlp_ps_cm.__enter__()

            x_sbs = {}

            def mlp_load(t):
                nt, col = N_TILES[t], cols[t]
                xs = xsp.tile([P, 2, 512], BF16, tag="xs")
                nc.sync.dma_start(
                    out=xs[:, :, :nt],
                    in_=xT[0:2 * P, col:col + nt].rearrange(
                        "(k p) n -> p k n", p=P))
                xa = xact.tile([P, 2, 512], BF16, tag="xa")
                nc.scalar.dma_start(
                    out=xa[:, :, :nt],
                    in_=xT[2 * P:4 * P, col:col + nt].rearrange(
                        "(k p) n -> p k n", p=P))
                x_sbs[t] = (xs, xa)

            def mlp_l1(t):
                nt, col = N_TILES[t], cols[t]
                xs, xa = x_sbs.pop(t)
                h1_ps = mlp_ps.tile([P, 512], F32, space="PSUM", tag="lin")
                for k in range(4):
                    rhs = xs[:, k, :nt] if k < 2 else xa[:, k - 2, :nt]
                    nc.tensor.matmul(out=h1_ps[:, :nt], lhsT=w1t_sb[k][:],
                                     rhs=rhs,
                                     start=(k == 0), stop=(k == 3))
                # relu+bias on DVE, freeing Act for layer 2
                h1_sb = h1p.tile([P, 512], BF16, tag="h1")
                nc.vector.tensor_scalar(
                    out=h1_sb[:, :nt], in0=h1_ps[:, :nt],
                    scalar1=b1_sb[:], scalar2=0.0,
                    op0=mybir.AluOpType.add, op1=mybir.AluOpType.max)
                h1_sbs[t] = h1_sb

            def mlp_l2(t):
                nt, col = N_TILES[t], cols[t]
                h2_ps = mlp_ps.tile([P, 512], F32, space="PSUM", tag="lin")
                nc.tensor.matmul(out=h2_ps[:, :nt], lhsT=w2t_sb[:],
                                 rhs=h1_sbs.pop(t)[:, :nt],
                                 start=True, stop=True)
                nc.scalar.activation(out=hA[:, col:col + nt], in_=h2_ps[:, :nt],
                                     func=mybir.ActivationFunctionType.Relu,
                                     bias=b2_sb[:])

            NTI = len(N_TILES)
            for t in range(NTI + 4):
                if t < NTI:
                    mlp_load(t)
                if 2 <= t < NTI + 2:
                    mlp_l1(t - 2)
                if 3 <= t < NTI + 3:
                    mlp_l2(t - 3)
                if t >= 4:
                    emit_transpose_tiles(hA, 0, cols[t - 4],
                                         cols[t - 4] + N_TILES[t - 4])
                    maybe_publish(0, t - 3)
            mlp_ps_cm.__exit__(None, None, None)
            ps_lin_cm = tc.tile_pool(name="ps_lin", bufs=2, space="PSUM")
            ps_lin = ps_lin_cm.__enter__()
            ps_agg_cm = tc.tile_pool(name="ps_agg", bufs=4, space="PSUM")
            ps_agg = ps_agg_cm.__enter__()

            emit_allgather(0)

            def conv_layer(layer, hT_in, hT_out, table, pub_idx=None,
                           tile_tail=None):
                icol = 0
                scol = 0
                done_tiles = 0
                evict_flip = 0
                tview = table[:, :].rearrange("(a two) h -> a two h", two=2)

                def emit_ready_linear(avail, done_tiles, force=False):
                    col = done_tiles * 512
                    while done_tiles < len(N_TILES):
                        nt = N_TILES[done_tiles]
                        if col + nt > avail and not force:
                            break
                        ps = ps_lin.tile([P, 512], F32, space="PSUM", tag="lin")
                        nc.tensor.matmul(out=ps[:, :nt], lhsT=cw_sb[layer][0][:],
                                         rhs=aggT[:, col:col + nt],
                                         start=True, stop=False)
                        nc.tensor.matmul(out=ps[:, :nt], lhsT=cw_sb[layer][1][:],
                                         rhs=hT_in[:, col:col + nt],
                                         start=False, stop=True)
                        nc.scalar.activation(
                            out=hT_out[:, col:col + nt], in_=ps[:, :nt],
                            func=mybir.ActivationFunctionType.Relu,
                            bias=cb_sb[layer][:])
                        if pub_idx is not None:
                            emit_transpose_tiles(hT_out, pub_idx, col, col + nt)
                        if tile_tail is not None:
                            tile_tail(col, nt)
                        col += nt
                        done_tiles += 1
                        if pub_idx is not None:
                            maybe_publish(pub_idx, done_tiles)
                    return done_tiles

                for ci, meta in enumerate(metas):
                    nbev, nbod = meta['nblk_ev'], meta['nblk_od']
                    nblk = nbev + nbod
                    nmm = sum(len(e) for _, e in meta['mms'])
                    s_w = swp.tile([P, MAXMM, W], BF16, tag="sw")
                    # the first two chunks' S_w prefetches ride the Pool DMA
                    # queue (idle during the MLP); later chunks stream on SP,
                    # paced by the double-buffered pool so the scheduler
                    # cannot hoist them into the MLP's x-tile loads
                    sw_eng = nc.gpsimd if (layer == 0 and ci < 2) else nc.sync
                    sw_eng.dma_start(
                        out=s_w[:, :nmm, :],
                        in_=esw[:, scol:scol + W * nmm].rearrange(
                            "p (m s) -> p m s", s=W))
                    msg = msgp.tile([P, MAXBLK, HID], BF16, tag="msg")
                    if nbev:
                        nc.gpsimd.dma_gather(
                            out_ap=msg[:, :nbev, :], in_ap=tview[:, 0, :],
                            idxs_ap=eidx_sb[:, icol:icol + nbev * 8],
                            num_idxs=nbev * P, num_idxs_reg=nbev * P,
                            elem_size=HID, elem_step=2 * HID,
                            single_packet=False, queue_num=0)
                    if nbod:
                        nc.gpsimd.dma_gather(
                            out_ap=msg[:, nbev:nblk, :], in_ap=tview[:, 1, :],
                            idxs_ap=eidx_sb[:, icol + nbev * 8:icol + nblk * 8],
                            num_idxs=nbod * P, num_idxs_reg=nbod * P,
                            elem_size=HID, elem_step=2 * HID,
                            single_packet=False, queue_num=0)

                    smi = 0
                    for g, ents in meta['mms']:
                        ps = ps_agg.tile([P, W], F32, space="PSUM", tag="agg")
                        gw = min(W, SHARD - g * W)
                        for i, (cat, b) in enumerate(ents):
                            bidx = b if cat == 'ev' else nbev + b
                            nc.tensor.matmul(out=ps[:, :gw],
                                             lhsT=msg[:, bidx, :],
                                             rhs=s_w[:, smi, :gw],
                                             start=(i == 0),
                                             stop=(i == len(ents) - 1))
                            smi += 1
                        # evictions alternate Act / DVE
                        if evict_flip == 0:
                            nc.scalar.activation(
                                out=aggT[:, g * W:g * W + gw], in_=ps[:, :gw],
                                func=mybir.ActivationFunctionType.Copy)
                        else:
                            nc.vector.tensor_copy(
                                out=aggT[:, g * W:g * W + gw], in_=ps[:, :gw])
                        evict_flip ^= 1
                    icol += nblk * 8
                    scol += W * nmm
                    done_tiles = emit_ready_linear(meta['groups'][1] * W,
                                                   done_tiles)
                done_tiles = emit_ready_linear(SHARD, done_tiles, force=True)

            conv_layer(0, hA, hB, tables[0], pub_idx=1)
            emit_allgather(1)

            # ---- heads fused into conv2's linear phase ----
            def head_tail(col, nt):
                ps = ps_lin.tile([2 * LAT, 512], F32, space="PSUM", tag="lin",
                                 name="headps")
                nc.tensor.matmul(out=ps[:, :nt], lhsT=hw_sb[:],
                                 rhs=hA[:, col:col + nt], start=True, stop=True)
                mst = stg.tile([2 * LAT, 512], F32, tag="mst")
                nc.vector.tensor_scalar(
                    out=mst[:, :nt], in0=ps[:, :nt], scalar1=hb_sb[:],
                    scalar2=None, op0=mybir.AluOpType.add)
                eng = nc.sync if (col // 512) % 2 == 0 else nc.scalar
                eng.dma_start(out=muv_out[:, col:col + nt], in_=mst[:, :nt])

            conv_layer(1, hB, hA, tables[1], tile_tail=head_tail)
            ps_agg_cm.__exit__(None, None, None)
            ps_lin_cm.__exit__(None, None, None)

    nc.finalize()
    return nc


# -------------------------------------------------------------------- driver --

def _get_compiled(x, edge_index, edge_attr, weights):
    src = np.asarray(edge_index[0]).astype(np.int64)
    dst = np.asarray(edge_index[1]).astype(np.int64)
    wgt = np.asarray(edge_attr, dtype=np.float32)
    x = np.asarray(x, dtype=np.float32)

    per_core_edges = []
    for c in range(N_CORES):
        sel = (dst >= c * SHARD) & (dst < (c + 1) * SHARD)
        s, d, wv = src[sel], dst[sel] - c * SHARD, wgt[sel]
        order = np.argsort(d, kind="stable")
        per_core_edges.append((s[order], d[order], wv[order]))

    metas, eidx, esw = _structure(per_core_edges)
    dims = dict(idx=eidx[0].shape[1], sw=esw[0].shape[1])

    nc = _build(metas, dims)

    (W1, b1, W2, b2, g1_rel_W, g1_rel_b, g1_root_W,
     g2_rel_W, g2_rel_b, g2_root_W, mu_W, mu_b, lv_W, lv_b) = [
        np.asarray(w, dtype=np.float32) for w in weights]

    conv_wT = np.stack([
        np.stack([g1_rel_W.T, g1_root_W.T]),
        np.stack([g2_rel_W.T, g2_root_W.T]),
    ]).astype(NP_BF16).copy()
    conv_b = np.stack([g1_rel_b[:, None], g2_rel_b[:, None]]).copy()
    headWT = np.ascontiguousarray(
        np.concatenate([mu_W, lv_W], axis=0).T.astype(NP_BF16))
    head_b = np.concatenate([mu_b, lv_b])[:, None].copy()

    common = dict(
        w1T=np.ascontiguousarray(W1.T.astype(NP_BF16)), b1=b1[:, None].copy(),
        w2T=np.ascontiguousarray(W2.T.astype(NP_BF16)), b2=b2[:, None].copy(),
        conv_wT=conv_wT, conv_b=conv_b, headWT=headWT, head_b=head_b,
    )
    in_maps = []
    for c in range(N_CORES):
        m = dict(common)
        m["xT"] = np.ascontiguousarray(x[c * SHARD:(c + 1) * SHARD, :].T.astype(NP_BF16))
        m["eidx"] = eidx[c]
        m["esw"] = esw[c]
        in_maps.append(m)
    return nc, in_maps


def kernel(x, edge_index, edge_attr,
           W1, b1, W2, b2,
           g1_rel_W, g1_rel_b, g1_root_W,
           g2_rel_W, g2_rel_b, g2_root_W,
           mu_W, mu_b, lv_W, lv_b):
    weights = (W1, b1, W2, b2, g1_rel_W, g1_rel_b, g1_root_W,
               g2_rel_W, g2_rel_b, g2_root_W, mu_W, mu_b, lv_W, lv_b)
    nc, in_maps = _get_compiled(x, edge_index, edge_attr, weights)
    res = bass_utils.run_bass_kernel_spmd(nc, in_maps,
                                          core_ids=list(range(N_CORES)))
    muvT = np.concatenate([res.results[c]["muvT"] for c in range(N_CORES)],
                          axis=1)
    return (np.ascontiguousarray(muvT[:LAT, :].T),
            np.ascontiguousarray(muvT[LAT:, :].T))


# revision 32
# speedup vs baseline: 1.0019x; 1.0019x over previous
"""DRASI encoder (MLP -> GraphConv x2 -> mu/logvar heads) on 8 Trainium2 cores.

Sharding: nodes are split into 8 contiguous shards of 6250. Each core runs the
node-local MLP on its shard, the shards are AllGathered into a full
[50000, 128] bf16 feature table in DRAM, and each core processes the edges
whose destination lies in its shard.

Aggregation strategy (both conv layers share the same edge structure):
  - edges are split by source-node parity (even/odd). The parity split is
    symmetric across cores, so unified (max-over-core) block counts carry
    minimal padding, and src//2 indices fit int16; each parity stream
    gathers through an elem_step=256 strided view of the table.
  - within each (chunk, parity) stream, edges are packed densely into
    128-edge gather blocks with no per-group padding; a block whose edges
    span a dst-group boundary contributes one masked matmul per group it
    touches. The selection matrices S_w[e, s] = w_e * (seg_e == s) are
    precomputed on the host in bf16 and streamed via plain DMA on the SP
    queue (which does not contend with the SWDGE gather path).
  - per-group PE matmuls accumulate aggT = msg.T @ S_w in PSUM (64-node
    groups) and evict to an SBUF table in bf16, alternating between the
    Act and DVE engines.
  - the GraphConv linear phase runs interleaved with aggregation: one PSUM
    group accumulates rel_W @ aggT + root_W @ h directly, one bias+relu
    activation per 512-column tile.
  - block counts and matmul sets are unified across cores (max/union) so all
    8 cores share one program.

Outputs (mu, logvar) are computed per shard and concatenated on the host.
"""
import sys
sys.path.insert(0, '/opt/trn_rl_repo')

import numpy as np
import concourse.bass as bass
import concourse.bacc as bacc
import concourse.mybir as mybir
from concourse.tile import TileContext
from concourse.masks import make_identity
from concourse import bass_utils

P = 128
N_CORES = 8
N_NODES = 50000
IN_DIM = 512
HID = 128
LAT = 32
SHARD = N_NODES // N_CORES          # 6250
HALF = N_NODES // 2                 # 25000
W = 64                              # nodes per segment group (PSUM tile width)
N_GROUPS = (SHARD + W - 1) // W     # 98 (last group 42 wide)
N_TILES = [512] * (SHARD // 512) + ([SHARD % 512] if SHARD % 512 else [])
F32 = mybir.dt.float32
BF16 = mybir.dt.bfloat16
I16 = mybir.dt.int16
import ml_dtypes
NP_BF16 = ml_dtypes.bfloat16


# ---------------------------------------------------------------- host prep --

def _idx_tile(idx_flat):
    """SWDGE index layout: [128, nblk*8] int16 from flat [nblk*128]."""
    nblk = idx_flat.shape[0] // P
    return np.tile(idx_flat.reshape(nblk * 8, 16).T, (8, 1))


def _structure(per_core_edges):
    """per_core_edges: list of (src_global, dst_local, w) sorted by dst_local.

    Returns (metas, eidx, esw); metas is a list of chunk dicts:
      groups: (gs, ge)
      nblk_ev, nblk_od
      mms: [(g, [(cat, b), ...]), ...] in emission order
    """
    cats = []           # per core: dict cat -> (gather idx, dstl, w)
    for c, (src, dstl, wgt) in enumerate(per_core_edges):
        mev = src % 2 == 0
        cats.append({
            'ev': (src[mev] // 2, dstl[mev], wgt[mev]),
            'od': (src[~mev] // 2, dstl[~mev], wgt[~mev]),
        })

    starts = [{k: np.searchsorted(v[1], np.arange(N_GROUPS + 1) * W)
               for k, v in cc.items()} for cc in cats]

    def nblk_range(cat, gs, ge):
        return max(int(np.ceil((starts[c][cat][ge] - starts[c][cat][gs]) / P))
                   for c in range(N_CORES))

    # greedy chunking with a decreasing tail so the post-gather drain
    # (last matmuls -> linear tiles -> publish, gating the AllGather) is
    # short; chunk size has no effect on the gather stream itself, which
    # runs back-to-back regardless
    def total_blocks(gs, ge):
        return nblk_range('ev', gs, ge) + nblk_range('od', gs, ge)

    whole = total_blocks(0, N_GROUPS)

    def cap_for(idx, emitted):
        rem = whole - emitted
        if rem > 150:
            return 84
        if rem > 110:
            return 60
        if rem > 70:
            return 40
        if rem > 40:
            return 24
        return 14

    chunks = []
    gs = 0
    emitted = 0
    for g in range(1, N_GROUPS + 1):
        cap = cap_for(len(chunks), emitted)
        nb = total_blocks(gs, g)
        if nb > cap and g - 1 > gs:
            chunks.append((gs, g - 1))
            emitted += total_blocks(gs, g - 1)
            gs = g - 1
    if gs < N_GROUPS:
        chunks.append((gs, N_GROUPS))

    metas = []
    idxs = [[] for _ in range(N_CORES)]
    sws = [[] for _ in range(N_CORES)]

    def cat_layout(cat, gs, ge):
        nblk = nblk_range(cat, gs, ge)
        mms = {}
        core_idx = []
        core_rows = []   # per core: dict (g, b) -> (slots, segs, ws)
        for c in range(N_CORES):
            st = starts[c][cat]
            base = st[gs]
            ids, dstl, ws = cats[c][cat]
            iflat = np.zeros(nblk * P, np.int16)
            n = st[ge] - base
            if n:
                iflat[:n] = ids[base:st[ge]].astype(np.int16)
            core_idx.append(iflat)
            rows = {}
            for g in range(gs, ge):
                e0, e1 = st[g] - base, st[g + 1] - base
                if e1 <= e0:
                    continue
                b0, b1 = e0 // P, (e1 - 1) // P
                mms.setdefault(g, set()).update(range(b0, b1 + 1))
                for b in range(b0, b1 + 1):
                    r0, r1 = max(e0, b * P), min(e1, (b + 1) * P)
                    sl = np.arange(r0, r1) - b * P
                    rows[(g, b)] = (sl,
                                    dstl[base + r0:base + r1] - g * W,
                                    ws[base + r0:base + r1])
            core_rows.append(rows)
        mms = {g: sorted(bs) for g, bs in mms.items()}
        return nblk, mms, core_idx, core_rows

    def sw_block(core_rows_c, g, b):
        sw = np.zeros((P, W), np.float32)
        ent = core_rows_c.get((g, b))
        if ent is not None:
            sl, segs, ws = ent
            sw[sl, segs] = ws
        return sw.astype(NP_BF16)

    for (gs, ge) in chunks:
        nbev, mmsev, idxev, rowsev = cat_layout('ev', gs, ge)
        nbod, mmsod, idxod, rowsod = cat_layout('od', gs, ge)

        gset = sorted(set(mmsev) | set(mmsod))
        mms = [(g, [('ev', b) for b in mmsev.get(g, [])]
                + [('od', b) for b in mmsod.get(g, [])]) for g in gset]

        metas.append(dict(groups=(gs, ge), nblk_ev=nbev, nblk_od=nbod,
                          mms=mms))

        for c in range(N_CORES):
            idxs[c].append(_idx_tile(np.concatenate([idxev[c], idxod[c]])))
            for g, ents in mms:
                for cat, b in ents:
                    sws[c].append(sw_block(
                        rowsev[c] if cat == 'ev' else rowsod[c], g, b))

    eidx = [np.ascontiguousarray(np.concatenate(idxs[c], axis=1))
            for c in range(N_CORES)]
    esw = [np.ascontiguousarray(np.concatenate(sws[c], axis=1))
           for c in range(N_CORES)]
    return metas, eidx, esw


# ------------------------------------------------------------- device build --

def _build(metas, dims):
    nc = bacc.Bacc(None, target_bir_lowering=False, num_devices=N_CORES,
                   num_swdge_queues=2)

    xT = nc.dram_tensor("xT", [IN_DIM, SHARD], BF16, kind="ExternalInput")
    w1T = nc.dram_tensor("w1T", [IN_DIM, HID], BF16, kind="ExternalInput")
    b1 = nc.dram_tensor("b1", [HID, 1], F32, kind="ExternalInput")
    w2T = nc.dram_tensor("w2T", [HID, HID], BF16, kind="ExternalInput")
    b2 = nc.dram_tensor("b2", [HID, 1], F32, kind="ExternalInput")
    conv_wT = nc.dram_tensor("conv_wT", [2, 2, HID, HID], BF16, kind="ExternalInput")
    conv_b = nc.dram_tensor("conv_b", [2, HID, 1], F32, kind="ExternalInput")
    headWT = nc.dram_tensor("headWT", [HID, 2 * LAT], BF16, kind="ExternalInput")
    head_b = nc.dram_tensor("head_b", [2 * LAT, 1], F32, kind="ExternalInput")
    eidx = nc.dram_tensor("eidx", [P, dims['idx']], I16, kind="ExternalInput")
    esw = nc.dram_tensor("esw", [P, dims['sw']], BF16, kind="ExternalInput")
    muv_out = nc.dram_tensor("muvT", [2 * LAT, SHARD], F32, kind="ExternalOutput")

    ag_in = [nc.dram_tensor(f"ag_in{i}", [SHARD, HID], BF16) for i in range(2)]
    tables = [nc.dram_tensor(f"h_full{i}", [N_NODES, HID], BF16,
                             addr_space="Shared") for i in range(2)]

    NT_FULL = SHARD // P            # 48 full 128-row publish tiles
    TAIL = SHARD - NT_FULL * P      # 106
    MAXBLK = max(m['nblk_ev'] + m['nblk_od'] for m in metas)
    MAXMM = max(sum(len(e) for _, e in m['mms']) for m in metas)

    with TileContext(nc) as tc:
        with (
            tc.tile_pool(name="const", bufs=1) as cp,
            tc.tile_pool(name="big", bufs=1) as bigp,
            tc.tile_pool(name="xsp", bufs=3) as xsp,
            tc.tile_pool(name="xact", bufs=3) as xact,
            tc.tile_pool(name="h1p", bufs=3) as h1p,
            tc.tile_pool(name="msgp", bufs=3) as msgp,
            tc.tile_pool(name="swp", bufs=2) as swp,
            tc.tile_pool(name="stg", bufs=3) as stg,
            tc.tile_pool(name="ps_tr", bufs=2, space="PSUM") as ps_tr,
        ):
            # ---- constants; MLP-critical ones on SP first (tiny), the rest
            # on the Pool DMA channel which is otherwise idle during the MLP
            w1t_sb = [cp.tile([P, HID], BF16, tag=f"w1_{k}", name=f"w1t_{k}")
                      for k in range(4)]
            b1_sb = cp.tile([P, 1], F32, tag="b1")
            b2_sb = cp.tile([P, 1], F32, tag="b2")
            w2t_sb = cp.tile([P, HID], BF16, tag="w2")
            # spread the MLP-critical consts across the three DMA queues so
            # the first x tiles aren't delayed
            nc.sync.dma_start(out=w1t_sb[0][:], in_=w1T[0:P, :])
            nc.sync.dma_start(out=b1_sb[:], in_=b1[:, :])
            nc.scalar.dma_start(out=w1t_sb[1][:], in_=w1T[P:2 * P, :])
            nc.gpsimd.dma_start(out=w1t_sb[2][:], in_=w1T[2 * P:3 * P, :])
            nc.gpsimd.dma_start(out=w1t_sb[3][:], in_=w1T[3 * P:4 * P, :])
            nc.gpsimd.dma_start(out=w2t_sb[:], in_=w2T[:, :])
            nc.gpsimd.dma_start(out=b2_sb[:], in_=b2[:, :])
            cw_sb = [[cp.tile([P, HID], BF16, tag=f"cw{l}{m}", name=f"cw_{l}_{m}")
                      for m in range(2)] for l in range(2)]
            cb_sb = [cp.tile([P, 1], F32, tag=f"cb{l}", name=f"cb_{l}")
                     for l in range(2)]
            for l in range(2):
                for m in range(2):
                    nc.gpsimd.dma_start(out=cw_sb[l][m][:], in_=conv_wT[l, m, :, :])
                nc.gpsimd.dma_start(out=cb_sb[l][:], in_=conv_b[l, :, :])
            hw_sb = cp.tile([P, 2 * LAT], BF16, tag="hw")
            nc.gpsimd.dma_start(out=hw_sb[:], in_=headWT[:, :])
            hb_sb = cp.tile([2 * LAT, 1], F32, tag="hb")
            nc.gpsimd.dma_start(out=hb_sb[:], in_=head_b[:, :])
            ident = cp.tile([P, P], BF16, tag="ident")
            make_identity(nc, ident[:])

            hA = bigp.tile([P, SHARD], BF16, tag="hA")   # h2T, then h4T
            hB = bigp.tile([P, SHARD], BF16, tag="hB")   # h3T
            aggT = bigp.tile([P, SHARD], BF16, tag="aggT")
            natf = [bigp.tile([P, NT_FULL, HID], BF16, tag=f"natf{i}",
                              name=f"natf_{i}") for i in range(2)]
            natt = [bigp.tile([P, HID], BF16, tag=f"natt{i}",
                              name=f"natt_{i}") for i in range(2)]
            eidx_sb = bigp.tile([P, dims['idx']], I16, tag="eidx")
            # edge index table: both layers reuse it; Pool channel is idle
            # during the MLP so the load is free there
            nc.gpsimd.dma_start(out=eidx_sb[:], in_=eidx[:, :])

            def emit_transpose_tiles(hT_tile, t_idx, n0, n1):
                t = n0 // P
                while n0 < n1:
                    w_ = min(P, n1 - n0)
                    tr_ps = ps_tr.tile([P, P], BF16, space="PSUM", tag="tr",
                                       name="trp")
                    nc.tensor.transpose(out=tr_ps[:w_, :],
                                        in_=hT_tile[:, n0:n0 + w_],
                                        identity=ident[:])
                    dst = natf[t_idx][:, t, :] if w_ == P else natt[t_idx][:TAIL, :]
                    nc.vector.tensor_copy(
                        out=dst[:w_, :] if w_ == P else dst,
                        in_=tr_ps[:w_, :])
                    n0 += w_
                    t += 1

            def emit_publish_piece(t_idx, c0, c1, eng):
                eng.dma_start(
                    out=ag_in[t_idx][c0 * P:c1 * P, :].rearrange(
                        "(t r) h -> r t h", r=P),
                    in_=natf[t_idx][:, c0:c1, :])
                if c1 == NT_FULL:
                    eng.dma_start(out=ag_in[t_idx][NT_FULL * P:, :],
                                  in_=natt[t_idx][:TAIL, :])

            PUB_AT = {4: (0, 16), 8: (16, 32), 11: (32, 40),
                      12: (40, 44), 13: (44, NT_FULL)}

            def maybe_publish(t_idx, done_tiles):
                if done_tiles in PUB_AT:
                    c0, c1 = PUB_AT[done_tiles]
                    # during the MLP (t_idx 0) SP is idle once the x tiles
                    # are in; during the conv layer (t_idx 1) SP streams S_w,
                    # so publishes ride Act instead (its channel is idle)
                    eng = nc.sync if t_idx == 0 else nc.scalar
                    emit_publish_piece(t_idx, c0, c1, eng)

            def emit_allgather(t_idx):
                nc.gpsimd.collective_compute(
                    "AllGather", mybir.AluOpType.bypass,
                    replica_groups=[list(range(N_CORES))],
                    ins=[ag_in[t_idx][:, :]],
                    outs=[tables[t_idx][:, :]],
                )

            # ---- MLP (bf16 matmuls, f32 psum), software-pipelined ----
            cols = [sum(N_TILES[:i]) for i in range(len(N_TILES))]
            h1_sbs = {}
            mlp_ps_cm = tc.tile_pool(name="mlp_ps", bufs=4, space="PSUM")
            mlp_ps = mlp_ps_cm.__enter__()

            x_sbs = {}

            def mlp_load(t):
                nt, col = N_TILES[t], cols[t]
                xs = xsp.tile([P, 2, 512], BF16, tag="xs")
                nc.sync.dma_start(
                    out=xs[:, :, :nt],
                    in_=xT[0:2 * P, col:col + nt].rearrange(
                        "(k p) n -> p k n", p=P))
                xa = xact.tile([P, 2, 512], BF16, tag="xa")
                nc.scalar.dma_start(
                    out=xa[:, :, :nt],
                    in_=xT[2 * P:4 * P, col:col + nt].rearrange(
                        "(k p) n -> p k n", p=P))
                x_sbs[t] = (xs, xa)

            def mlp_l1(t):
                nt, col = N_TILES[t], cols[t]
                xs, xa = x_sbs.pop(t)
                h1_ps = mlp_ps.tile([P, 512], F32, space="PSUM", tag="lin")
                for k in range(4):
                    rhs = xs[:, k, :nt] if k < 2 else xa[:, k - 2, :nt]
                    nc.tensor.matmul(out=h1_ps[:, :nt], lhsT=w1t_sb[k][:],
                                     rhs=rhs,
                                     start=(k == 0), stop=(k == 3))
                # relu+bias on DVE, freeing Act for layer 2
                h1_sb = h1p.tile([P, 512], BF16, tag="h1")
                nc.vector.tensor_scalar(
                    out=h1_sb[:, :nt], in0=h1_ps[:, :nt],
                    scalar1=b1_sb[:], scalar2=0.0,
                    op0=mybir.AluOpType.add, op1=mybir.AluOpType.max)
                h1_sbs[t] = h1_sb

            def mlp_l2(t):
                nt, col = N_TILES[t], cols[t]
                h2_ps = mlp_ps.tile([P, 512], F32, space="PSUM", tag="lin")
                nc.tensor.matmul(out=h2_ps[:, :nt], lhsT=w2t_sb[:],
                                 rhs=h1_sbs.pop(t)[:, :nt],
                                 start=True, stop=True)
                nc.scalar.activation(out=hA[:, col:col + nt], in_=h2_ps[:, :nt],
                                     func=mybir.ActivationFunctionType.Relu,
                                     bias=b2_sb[:])

            NTI = len(N_TILES)
            for t in range(NTI + 4):
                if t < NTI:
                    mlp_load(t)
                if 2 <= t < NTI + 2:
                    mlp_l1(t - 2)
                if 3 <= t < NTI + 3:
                    mlp_l2(t - 3)
                if t >= 4:
                    emit_transpose_tiles(hA, 0, cols[t - 4],
                                         cols[t - 4] + N_TILES[t - 4])
                    maybe_publish(0, t - 3)
            mlp_ps_cm.__exit__(None, None, None)
            ps_lin_cm = tc.tile_pool(name="ps_lin", bufs=2, space="PSUM")
            ps_lin = ps_lin_cm.__enter__()
            ps_agg_cm = tc.tile_pool(name="ps_agg", bufs=4, space="PSUM")
            ps_agg = ps_agg_cm.__enter__()

            emit_allgather(0)

            def conv_layer(layer, hT_in, hT_out, table, pub_idx=None,
                           tile_tail=None):
                icol = 0
                scol = 0
                done_tiles = 0
                evict_flip = 0
                tview = table[:, :].rearrange("(a two) h -> a two h", two=2)

                def emit_ready_linear(avail, done_tiles, force=False):
                    col = done_tiles * 512
                    while done_tiles < len(N_TILES):
                        nt = N_TILES[done_tiles]
                        if col + nt > avail and not force:
                            break
                        ps = ps_lin.tile([P, 512], F32, space="PSUM", tag="lin")
                        nc.tensor.matmul(out=ps[:, :nt], lhsT=cw_sb[layer][0][:],
                                         rhs=aggT[:, col:col + nt],
                                         start=True, stop=False)
                        nc.tensor.matmul(out=ps[:, :nt], lhsT=cw_sb[layer][1][:],
                                         rhs=hT_in[:, col:col + nt],
                                         start=False, stop=True)
                        nc.scalar.activation(
                            out=hT_out[:, col:col + nt], in_=ps[:, :nt],
                            func=mybir.ActivationFunctionType.Relu,
                            bias=cb_sb[layer][:])
                        if pub_idx is not None:
                            emit_transpose_tiles(hT_out, pub_idx, col, col + nt)
                        if tile_tail is not None:
                            tile_tail(col, nt)
                        col += nt
                        done_tiles += 1
                        if pub_idx is not None:
                            maybe_publish(pub_idx, done_tiles)
                    return done_tiles

                for ci, meta in enumerate(metas):
                    nbev, nbod = meta['nblk_ev'], meta['nblk_od']
                    nblk = nbev + nbod
                    nmm = sum(len(e) for _, e in meta['mms'])
                    s_w = swp.tile([P, MAXMM, W], BF16, tag="sw")
                    # the first two chunks' S_w prefetches ride the Pool DMA
                    # queue (idle during the MLP); later chunks stream on SP,
                    # paced by the double-buffered pool so the scheduler
                    # cannot hoist them into the MLP's x-tile loads
                    sw_eng = nc.gpsimd if (layer == 0 and ci < 2) else nc.sync
                    sw_eng.dma_start(
                        out=s_w[:, :nmm, :],
                        in_=esw[:, scol:scol + W * nmm].rearrange(
                            "p (m s) -> p m s", s=W))
                    msg = msgp.tile([P, MAXBLK, HID], BF16, tag="msg")
                    if nbev:
                        nc.gpsimd.dma_gather(
                            out_ap=msg[:, :nbev, :], in_ap=tview[:, 0, :],
                            idxs_ap=eidx_sb[:, icol:icol + nbev * 8],
                            num_idxs=nbev * P, num_idxs_reg=nbev * P,
                            elem_size=HID, elem_step=2 * HID,
                            single_packet=False, queue_num=0)
                    if nbod:
                        nc.gpsimd.dma_gather(
                            out_ap=msg[:, nbev:nblk, :], in_ap=tview[:, 1, :],
                            idxs_ap=eidx_sb[:, icol + nbev * 8:icol + nblk * 8],
                            num_idxs=nbod * P, num_idxs_reg=nbod * P,
                            elem_size=HID, elem_step=2 * HID,
                            single_packet=False, queue_num=0)

                    smi = 0
                    for g, ents in meta['mms']:
                        ps = ps_agg.tile([P, W], F32, space="PSUM", tag="agg")
                        gw = min(W, SHARD - g * W)
                        for i, (cat, b) in enumerate(ents):
                            bidx = b if cat == 'ev' else nbev + b
                            nc.tensor.matmul(out=ps[:, :gw],
                                             lhsT=msg[:, bidx, :],
                                             rhs=s_w[:, smi, :gw],
                                             start=(i == 0),
                                             stop=(i == len(ents) - 1))
                            smi += 1
                        # evictions: 2 of 3 on DVE (Act also carries the
                        # linear relus and the publishes)
                        if evict_flip == 0:
                            nc.scalar.activation(
                                out=aggT[:, g * W:g * W + gw], in_=ps[:, :gw],
                                func=mybir.ActivationFunctionType.Copy)
                        else:
                            nc.vector.tensor_copy(
                                out=aggT[:, g * W:g * W + gw], in_=ps[:, :gw])
                        evict_flip = (evict_flip + 1) % 3
                    icol += nblk * 8
                    scol += W * nmm
                    done_tiles = emit_ready_linear(meta['groups'][1] * W,
                                                   done_tiles)
                done_tiles = emit_ready_linear(SHARD, done_tiles, force=True)

            conv_layer(0, hA, hB, tables[0], pub_idx=1)
            emit_allgather(1)

            # ---- heads fused into conv2's linear phase ----
            def head_tail(col, nt):
                ps = ps_lin.tile([2 * LAT, 512], F32, space="PSUM", tag="lin",
                                 name="headps")
                nc.tensor.matmul(out=ps[:, :nt], lhsT=hw_sb[:],
                                 rhs=hA[:, col:col + nt], start=True, stop=True)
                mst = stg.tile([2 * LAT, 512], F32, tag="mst")
                nc.vector.tensor_scalar(
                    out=mst[:, :nt], in0=ps[:, :nt], scalar1=hb_sb[:],
                    scalar2=None, op0=mybir.AluOpType.add)
                nc.scalar.dma_start(out=muv_out[:, col:col + nt],
                                    in_=mst[:, :nt])

            conv_layer(1, hB, hA, tables[1], tile_tail=head_tail)
            ps_agg_cm.__exit__(None, None, None)
            ps_lin_cm.__exit__(None, None, None)

    nc.finalize()
    return nc


# -------------------------------------------------------------------- driver --

def _get_compiled(x, edge_index, edge_attr, weights):
    src = np.asarray(edge_index[0]).astype(np.int64)
    dst = np.asarray(edge_index[1]).astype(np.int64)
    wgt = np.asarray(edge_attr, dtype=np.float32)
    x = np.asarray(x, dtype=np.float32)

    per_core_edges = []
    for c in range(N_CORES):
        sel = (dst >= c * SHARD) & (dst < (c + 1) * SHARD)
        s, d, wv = src[sel], dst[sel] - c * SHARD, wgt[sel]
        order = np.argsort(d, kind="stable")
        per_core_edges.append((s[order], d[order], wv[order]))

    metas, eidx, esw = _structure(per_core_edges)
    dims = dict(idx=eidx[0].shape[1], sw=esw[0].shape[1])

    nc = _build(metas, dims)

    (W1, b1, W2, b2, g1_rel_W, g1_rel_b, g1_root_W,
     g2_rel_W, g2_rel_b, g2_root_W, mu_W, mu_b, lv_W, lv_b) = [
        np.asarray(w, dtype=np.float32) for w in weights]

    conv_wT = np.stack([
        np.stack([g1_rel_W.T, g1_root_W.T]),
        np.stack([g2_rel_W.T, g2_root_W.T]),
    ]).astype(NP_BF16).copy()
    conv_b = np.stack([g1_rel_b[:, None], g2_rel_b[:, None]]).copy()
    headWT = np.ascontiguousarray(
        np.concatenate([mu_W, lv_W], axis=0).T.astype(NP_BF16))
    head_b = np.concatenate([mu_b, lv_b])[:, None].copy()

    common = dict(
        w1T=np.ascontiguousarray(W1.T.astype(NP_BF16)), b1=b1[:, None].copy(),
        w2T=np.ascontiguousarray(W2.T.astype(NP_BF16)), b2=b2[:, None].copy(),
        conv_wT=conv_wT, conv_b=conv_b, headWT=headWT, head_b=head_b,
    )
    in_maps = []
    for c in range(N_CORES):
        m = dict(common)
        m["xT"] = np.ascontiguousarray(x[c * SHARD:(c + 1) * SHARD, :].T.astype(NP_BF16))
        m["eidx"] = eidx[c]
        m["esw"] = esw[c]
        in_maps.append(m)
    return nc, in_maps


def kernel(x, edge_index, edge_attr,
           W1, b1, W2, b2,
           g1_rel_W, g1_rel_b, g1_root_W,
           g2_rel_W, g2_rel_b, g2_root_W,
           mu_W, mu_b, lv_W, lv_b):
    weights = (W1, b1, W2, b2, g1_rel_W, g1_rel_b, g1_root_W,
               g2_rel_W, g2_rel_b, g2_root_W, mu_W, mu_b, lv_W, lv_b)
    nc, in_maps = _get_compiled(x, edge_index, edge_attr, weights)
    res = bass_utils.run_bass_kernel_spmd(nc, in_maps,
                                          core_ids=list(range(N_CORES)))
    muvT = np.concatenate([res.results[c]["muvT"] for c in range(N_CORES)],
                          axis=1)
    return (np.ascontiguousarray(muvT[:LAT, :].T),
            np.ascontiguousarray(muvT[LAT:, :].T))
